# Initial kernel scaffold
#
"""LLaDA2 MoE decoder layer as a single SPMD Bass/Tile kernel on 8 TRN2 cores.

v2 sharding — minimize collective traffic (the baseline spent ~440us of its
1346us inside ReduceScatter/AllGather at ~30-60 GB/s):
  - Attention: core c handles batch b = c//4, kv-group kv = c%4 (1 kv head +
    its 4 q heads, full 1024-token sequence). A small 4-core AllToAll
    (wire ~1.5MB/core) redistributes attention outputs so that each core ends
    up with the full 2048-dim attention output for its OWN 256 tokens, in
    transposed layout. The dense projection then runs locally with a fully
    replicated w_dense (weight DMA is cheap and overlaps with attention).
  - MoE: expert-parallel degree 2. Cores (2p, 2p+1) pool their 512 tokens:
    a pair AllGather shares x2^T + routing rows; core 2p runs experts 0-3,
    core 2p+1 runs experts 4-7 (selection via a per-core one-hot input so the
    program stays strictly SPMD), each over all 512 pair tokens, dense,
    route-masked, all-expert partials accumulated in PSUM. A pair AllToAll
    returns per-token partials; each owner sums the two blocks locally.
  - Shared expert runs locally on own tokens while the pair AllGather is in
    flight. Expert weights stream on the scalar-engine HWDGE ring so they
    never block latency-critical activation DMAs on the sync ring.

All matmuls are f32r with fp32 PSUM accumulation (same numeric path as the
accepted baseline); moving dims are >=256 everywhere so f32r streams at full
PE rate. Reciprocals/rsqrts on [1,N] rows use exp(-ln x) on the scalar engine
(the DVE reciprocal is ~6.5us per row and serialized the baseline's softmax).
"""

import math

import numpy as np

import concourse.bass as bass
import concourse.mybir as mybir
import concourse.tile as tile
from concourse import bacc
from concourse.masks import make_identity

# Problem shapes (hardcoded per contest rules).
B, S, H = 2, 1024, 2048
NH, NKV, HD = 16, 4, 128
E, TOPK, NG, TOPKG, M = 8, 2, 4, 2, 512
EPS = 1e-6
SCALE = HD ** -0.5
T = B * S
NCORES = 8
TOK = T // NCORES              # 256 tokens owned per core
QH = NH // NKV                 # 4 q heads per kv head
P = 128
F32 = mybir.dt.float32
F32R = mybir.dt.float32r
BIG = 1e30

HC = H // P                    # 16 h chunks
SC = S // P                    # 8 seq chunks per batch
MC = M // P                    # 4 m chunks
TC = TOK // P                  # 2 own-token chunks
NT = H // 512                  # 4 512-wide h' tiles
EL = 4                         # experts per core (EP=2 over pairs)
PTOK = 2 * TOK                 # 512 tokens per pair
PTC = PTOK // P                # 4 pair-token chunks
AGROWS = H + E                 # 2056 rows in pair-allgather buffer
OC = NH * HD // P              # 16 o chunks (attention output dim)


def r32(ap):
    return ap.bitcast(F32R)


def _mm(nc, out, lhsT, rhs, start, stop):
    nc.tensor.matmul(out, r32(lhsT), r32(rhs), start=start, stop=stop)


def _dma_a(nc, out, in_):
    # latency-critical / activation DMAs: sync (SP) HWDGE ring
    nc.sync.dma_start(out=out.bitcast(F32R), in_=in_.bitcast(F32R))


def _dma_w(nc, out, in_):
    # bulk weight streams: scalar (ACT) HWDGE ring, so they never queue in
    # front of critical activation transfers
    nc.scalar.dma_start(out=out.bitcast(F32R), in_=in_.bitcast(F32R))


def build_program():
    nc = bacc.Bacc("TRN2", target_bir_lowering=False, debug=False,
                   num_devices=NCORES)

    def inp(name, shape):
        return nc.dram_tensor(name, list(shape), F32, kind="ExternalInput").ap()

    hT = inp("hT", (H, S))                 # hidden[b].T
    hid_own = inp("hid_own", (TOK, H))     # own-token hidden slice (natural)
    cos_qw = inp("cos_qw", (HD, S))        # cos[b].T * q_ln_w
    sin_qw = inp("sin_qw", (HD, S))        # sin[b].T * q_ln_w[perm] * sign
    cos_kw = inp("cos_kw", (HD, S))
    sin_kw = inp("sin_kw", (HD, S))
    wqkvT = inp("wqkvT", (H, 6 * P))       # [h, 4q+k+v heads] * ln1_w fold
    wdTo = inp("wdTo", (NH * HD, H))       # w_dense.T (full)
    gate_wT = inp("gate_wT", (H, E))       # gate_w.T * ln2_w fold
    gate_b = inp("gate_b", (1, E))
    esel = inp("esel", (E, EL))            # one-hot cols of this core's experts
    wg_all = inp("wg_all", (EL * H, M))    # per-expert we_gate[e].T * ln2 fold
    wu_all = inp("wu_all", (EL * H, M))
    wdn_all = inp("wdn_all", (EL * M, H))  # per-expert we_down[e].T
    wsgT = inp("wsgT", (H, M))             # shared expert, ln2_w folded
    wsuT = inp("wsuT", (H, M))
    wsdT = inp("wsdT", (M, H))
    mlo = inp("mlo", (P, 1))               # 1.0 on batch-0 cores else 0.0
    mhi = inp("mhi", (P, 1))               # 1.0 on batch-1 cores else 0.0

    out = nc.dram_tensor("out", [TOK, H], F32, kind="ExternalOutput").ap()

    ACT = mybir.ActivationFunctionType
    LNS = float(math.log(SCALE))

    with tile.TileContext(nc) as tc:
        with (
            tc.tile_pool(name="dram", bufs=1, space="DRAM") as dram,
            tc.tile_pool(name="const", bufs=1) as const,
            tc.tile_pool(name="ps_row", bufs=2, space="PSUM") as ps_row,
            tc.tile_pool(name="ps_tr", bufs=2, space="PSUM") as ps_tr,
            tc.tile_pool(name="ps_mm", bufs=4, space="PSUM") as ps_mm,
        ):
            # DRAM bounce buffers for collectives
            a2a1_in = [dram.tile([NH * HD, TOK], F32, name=f"a2a1i{i}")
                       for i in range(2)]
            a2a1_out = [dram.tile([NH * HD, TOK], F32, name=f"a2a1o{i}")
                        for i in range(2)]
            ag_in = dram.tile([AGROWS, TOK], F32)
            ag_out = dram.tile([2 * AGROWS, TOK], F32)
            rs2_in = [dram.tile([PTOK, H // 2], F32, name=f"rs2i{i}")
                      for i in range(2)]
            rs2_out = [dram.tile([TOK, H // 2], F32, name=f"rs2o{i}")
                       for i in range(2)]
            r1_dram = dram.tile([1, S], F32)

            cstage = const.tile([P, 1], F32)
            cstage_r = const.tile([1, P], F32)
            ones_col = const.tile([P, 1], F32)
            nc.vector.memset(cstage, 1.0)
            nc.scalar.activation(ones_col.bitcast(F32R), cstage, ACT.Copy)
            ones_row = const.tile([1, P], F32)
            nc.vector.memset(cstage_r, 1.0)
            nc.scalar.activation(ones_row.bitcast(F32R), cstage_r, ACT.Copy)
            invH_col = const.tile([P, 1], F32)
            nc.vector.memset(cstage, 1.0 / H)
            nc.scalar.activation(invH_col.bitcast(F32R), cstage, ACT.Copy)
            invHD_col = const.tile([P, 1], F32)
            nc.vector.memset(cstage, 1.0 / HD)
            nc.scalar.activation(invHD_col.bitcast(F32R), cstage, ACT.Copy)
            epsc = const.tile([P, 1], F32)
            nc.vector.memset(epsc, EPS)
            lnsc = const.tile([P, 1], F32)
            nc.vector.memset(lnsc, LNS)
            zeroc = const.tile([P, 1], F32)
            nc.vector.memset(zeroc, 0.0)
            ident = const.tile([P, P], F32)
            make_identity(nc, ident)

            # ============== ATTENTION (batch b, kv group kv) ==============
            with (
                tc.tile_pool(name="att_keep", bufs=1) as att_keep,
            ):
                qk_tiles = [att_keep.tile([P, S], F32, tag=f"qk{i}",
                                          name=f"qk{i}") for i in range(5)]
                vT = att_keep.tile([P, S], F32, tag="vT", name="vT")
                v_nat = [att_keep.tile([P, P], F32, tag=f"vn{i}",
                                       name=f"vn{i}") for i in range(SC)]
                r1 = att_keep.tile([1, S], F32, tag="r1", name="r1")
                r1T = att_keep.tile([P, SC], F32, tag="r1T", name="r1T")
                r_heads = [att_keep.tile([1, S], F32, tag=f"r_head{i}",
                                         name=f"r_head{i}")
                           for i in range(5)]
                oT = [att_keep.tile([P, S], F32, tag=f"oT{i}",
                                    name=f"oT{i}") for i in range(QH)]

                # ---- Phase A+B: r1, qkv projections (hT resident) ----
                with (
                    tc.tile_pool(name="ab_h", bufs=4) as ab_h,
                    tc.tile_pool(name="ab_w", bufs=4) as ab_w,
                    tc.tile_pool(name="ab_sq", bufs=2) as ab_sq,
                ):
                    # one big strided DMA per half-matrix: far fewer queue
                    # triggers and bigger bursts for the 16 SDMA engines
                    wqa = []
                    for rh in range(4):
                        twq = ab_w.tile([P, 4, 6 * P], F32, tag="wq")
                        _dma_w(nc, twq,
                               wqkvT[rh * 512:(rh + 1) * 512, :]
                               .rearrange("(c p) o -> p c o", p=P))
                        wqa.append(twq)
                    w_tiles = [wqa[hc // 4][:, hc % 4, :] for hc in range(HC)]
                    hta = []
                    for rh in range(4):
                        th = ab_h.tile([P, 4, S], F32, tag="hT")
                        _dma_a(nc, th,
                               hT[rh * 512:(rh + 1) * 512, :]
                               .rearrange("(c p) s -> p c s", p=P))
                        hta.append(th)
                    h_tiles = [hta[hc // 4][:, hc % 4, :] for hc in range(HC)]

                    ps_r1 = [ps_row.tile([1, 512], F32, tag="row1",
                                         name=f"psr1_{_i}")
                             for _i in range(2)]
                    for hc in range(HC):
                        sq = ab_sq.tile([P, S], F32, tag="sq")
                        nc.vector.tensor_mul(sq.bitcast(F32R), h_tiles[hc],
                                             h_tiles[hc])
                        for t2 in range(2):
                            _mm(nc, ps_r1[t2],
                                invH_col, sq[:, t2 * 512:(t2 + 1) * 512],
                                start=(hc == 0), stop=(hc == HC - 1))
                    # r1 = (mean + eps)^-0.5 via exp(-0.5 ln(mean + eps))
                    lnr = ab_sq.tile([1, S], F32, tag="lnr")
                    for t2 in range(2):
                        nc.scalar.activation(lnr[:, t2 * 512:(t2 + 1) * 512],
                                             ps_r1[t2], ACT.Ln,
                                             bias=epsc[0:1])
                    nc.scalar.activation(r1, lnr, ACT.Exp, scale=-0.5)
                    nc.sync.dma_start(out=r1_dram, in_=r1)
                    nc.sync.dma_start(
                        out=r1T,
                        in_=r1_dram.rearrange("o (a p) -> (o p) a", p=P))

                    for oc in range(6):
                        dst = qk_tiles[oc] if oc < 5 else vT
                        for t2 in range(2):
                            pq = ps_mm.tile([P, 512], F32, tag="mm")
                            for hc in range(HC):
                                _mm(nc, pq,
                                    w_tiles[hc][:, oc * P:(oc + 1) * P],
                                    h_tiles[hc][:, t2 * 512:(t2 + 1) * 512],
                                    start=(hc == 0), stop=(hc == HC - 1))
                            dslc = dst[:, t2 * 512:(t2 + 1) * 512]
                            if oc < 5:
                                dslc = dslc.bitcast(F32R)
                            nc.scalar.activation(dslc, pq, ACT.Copy)

                # v: PE-transpose to natural [s, d], scaled by r1
                for sc in range(SC):
                    pt = ps_tr.tile([P, P], F32, tag="tr")
                    nc.tensor.transpose(pt, vT[:, sc * P:(sc + 1) * P], ident)
                    nc.scalar.activation(v_nat[sc].bitcast(F32R), pt,
                                         ACT.Copy, scale=r1T[:, sc:sc + 1])

                # ---- Phase C: q_ln / k_ln rms factors (5 heads) ----
                with tc.tile_pool(name="c_tmp", bufs=2) as c_tmp:
                    for hh in range(5):
                        sq = c_tmp.tile([P, S], F32, tag="sqc")
                        nc.vector.tensor_mul(sq.bitcast(F32R), qk_tiles[hh],
                                             qk_tiles[hh])
                        lnt = c_tmp.tile([1, S], F32, tag="lnt")
                        for t2 in range(2):
                            ps_rh = ps_row.tile([1, 512], F32, tag="row1")
                            _mm(nc, ps_rh,
                                invHD_col, sq[:, t2 * 512:(t2 + 1) * 512],
                                start=True, stop=True)
                            nc.scalar.activation(
                                lnt[:, t2 * 512:(t2 + 1) * 512],
                                ps_rh, ACT.Ln, bias=epsc[0:1])
                        # q heads get softmax 1/sqrt(HD) folded in via bias
                        nc.scalar.activation(
                            r_heads[hh].bitcast(F32R), lnt, ACT.Exp,
                            scale=-0.5,
                            bias=(lnsc[0:1] if hh < 4 else zeroc[0:1]))

                # ---- Phase D: rope (in-place into qk tiles) ----
                with (
                    tc.tile_pool(name="d_cs", bufs=1) as d_cs,
                    tc.tile_pool(name="d_tmp", bufs=2) as d_tmp,
                ):
                    cq = d_cs.tile([P, S], F32, tag="cq", name="cq")
                    sq_ = d_cs.tile([P, S], F32, tag="sq_", name="sq_")
                    ck = d_cs.tile([P, S], F32, tag="ck", name="ck")
                    sk = d_cs.tile([P, S], F32, tag="sk", name="sk")
                    nc.sync.dma_start(out=cq, in_=cos_qw[:, :])
                    nc.sync.dma_start(out=sq_, in_=sin_qw[:, :])
                    nc.sync.dma_start(out=ck, in_=cos_kw[:, :])
                    nc.sync.dma_start(out=sk, in_=sin_kw[:, :])
                    for hh in range(5):
                        cw, sw = (cq, sq_) if hh < 4 else (ck, sk)
                        src = qk_tiles[hh]
                        swp = d_tmp.tile([P, S], F32, tag="swp")
                        nc.sync.dma_start(out=swp[0:64, :], in_=src[64:128, :])
                        nc.sync.dma_start(out=swp[64:128, :], in_=src[0:64, :])
                        ta = d_tmp.tile([P, S], F32, tag="ropeA")
                        nc.vector.tensor_mul(ta, src, cw)
                        nc.vector.tensor_mul(swp, swp, sw)
                        nc.vector.tensor_add(ta, ta, swp)
                        for t2 in range(2):
                            pb = ps_mm.tile([P, 512], F32, tag="mm",
                                            name=f"pbr{hh}_{t2}")
                            _mm(nc, pb, ones_row,
                                r_heads[hh][:, t2 * 512:(t2 + 1) * 512],
                                start=True, stop=True)
                            nc.vector.tensor_mul(
                                src[:, t2 * 512:(t2 + 1) * 512].bitcast(F32R),
                                ta[:, t2 * 512:(t2 + 1) * 512], pb)

                # ---- Phase E: attention per (head, t2) ----
                # Heads are processed in pairs; each pair's outputs are staged
                # and its half-AllToAll launched while the next pair computes,
                # hiding most of the exchange behind attention itself.
                kT = qk_tiles[4]
                mlo_t = att_keep.tile([P, 1], F32, tag="mlo", name="mlo")
                mhi_t = att_keep.tile([P, 1], F32, tag="mhi", name="mhi")
                _dma_a(nc, mlo_t, mlo[:, :])
                _dma_a(nc, mhi_t, mhi[:, :])
                with (
                    tc.tile_pool(name="att_exp", bufs=4) as att_exp,
                    tc.tile_pool(name="att_dt", bufs=8) as att_dt,
                    tc.tile_pool(name="att_row", bufs=4) as att_row,
                    tc.tile_pool(name="a2st", bufs=2) as a2st,
                ):
                  for hp in range(2):
                    for hh in (2 * hp, 2 * hp + 1):
                        qT = qk_tiles[hh]
                        for t2 in range(2):
                            ex = att_exp.tile([P, SC, 512], F32, tag="exp")
                            for sc in range(SC):
                                pst = ps_mm.tile([P, 512], F32, tag="mm")
                                _mm(nc, pst, kT[:, sc * P:(sc + 1) * P],
                                    qT[:, t2 * 512:(t2 + 1) * 512],
                                    start=True, stop=True)
                                nc.scalar.activation(
                                    ex[:, sc, :].bitcast(F32R), pst, ACT.Exp)
                            # denominator: DVE tree-add then one row matmul
                            s4 = [att_dt.tile([P, 512], F32, tag="dt",
                                              name=f"dt{hh}_{t2}_{i}")
                                  for i in range(4)]
                            for i in range(4):
                                nc.vector.tensor_add(
                                    s4[i].bitcast(F32R),
                                    ex[:, 2 * i, :], ex[:, 2 * i + 1, :])
                            nc.vector.tensor_add(s4[0].bitcast(F32R),
                                                 s4[0], s4[1])
                            nc.vector.tensor_add(s4[2].bitcast(F32R),
                                                 s4[2], s4[3])
                            nc.vector.tensor_add(s4[0].bitcast(F32R),
                                                 s4[0], s4[2])
                            ps_den = ps_row.tile([1, 512], F32, tag="row1")
                            _mm(nc, ps_den, ones_col, s4[0],
                                start=True, stop=True)
                            # 1/den = exp(-ln(den))
                            lnd = att_row.tile([1, 512], F32, tag="lnd")
                            nc.scalar.activation(lnd, ps_den, ACT.Ln)
                            rden = att_row.tile([1, 512], F32, tag="rden")
                            nc.scalar.activation(rden.bitcast(F32R), lnd,
                                                 ACT.Exp, scale=-1.0)
                            pb = ps_mm.tile([P, 512], F32, tag="mm")
                            _mm(nc, pb, ones_row, rden, start=True, stop=True)
                            rden_b = att_row.tile([P, 512], F32, tag="rden_b")
                            nc.scalar.activation(rden_b, pb, ACT.Copy)
                            po = ps_mm.tile([P, 512], F32, tag="mm")
                            for sc in range(SC):
                                _mm(nc, po, v_nat[sc], ex[:, sc, :],
                                    start=(sc == 0), stop=(sc == SC - 1))
                            nc.vector.tensor_mul(
                                oT[hh][:, t2 * 512:(t2 + 1) * 512]
                                .bitcast(F32R), po, rden_b)

                    # stage this head-pair for its half-AllToAll. AllToAll
                    # only supports the full 8-core mesh, but each core only
                    # has data for its 3 same-batch peers. Trick: stage the
                    # slab for same-batch peer slot k at BOTH block k and
                    # block k+4, pre-scaled by the mlo/mhi (1/0 per-batch)
                    # masks. Receivers sum blocks i and i+4: the cross-batch
                    # contribution is exactly zero, the same-batch one
                    # survives — every address stays rank-independent (SPMD).
                    for k in range(NKV):
                        for lh in range(2):
                            hh = 2 * hp + lh
                            slab = oT[hh][:, k * TOK:(k + 1) * TOK]
                            lo = a2st.tile([P, TOK], F32, tag="stlo")
                            nc.scalar.activation(lo, slab, ACT.Copy,
                                                 scale=mlo_t)
                            nc.scalar.dma_start(
                                out=a2a1_in[hp][k * 256 + lh * P:
                                                k * 256 + (lh + 1) * P, :],
                                in_=lo)
                            hi = a2st.tile([P, TOK], F32, tag="sthi")
                            nc.scalar.activation(hi, slab, ACT.Copy,
                                                 scale=mhi_t)
                            nc.scalar.dma_start(
                                out=a2a1_in[hp][1024 + k * 256 + lh * P:
                                                1024 + k * 256 +
                                                (lh + 1) * P, :],
                                in_=hi)
                    nc.gpsimd.collective_compute(
                        "AllToAll", mybir.AluOpType.bypass,
                        replica_groups=[[0, 1, 2, 3, 4, 5, 6, 7]],
                        ins=[a2a1_in[hp].opt()], outs=[a2a1_out[hp].opt()])

            # ============ dense proj + ln2 + routing (own 256 tokens) ======
            from contextlib import ExitStack as _ES
            _x2_ctx = _ES()
            with (
                tc.tile_pool(name="keepG", bufs=1) as keepG,
            ):
                hidden2 = [keepG.tile([P, H], F32, tag=f"h2_{i}",
                                      name=f"h2_{i}") for i in range(TC)]
                x2keep = _x2_ctx.enter_context(
                    tc.tile_pool(name="x2keep", bufs=1))
                x2T_own = [x2keep.tile([P, TOK], F32, tag=f"x2o{i}",
                                       name=f"x2o{i}") for i in range(HC)]

                with (
                    tc.tile_pool(name="wdto", bufs=2) as wdto_pool,
                    tc.tile_pool(name="otf", bufs=OC) as otf_pool,
                    tc.tile_pool(name="otbig", bufs=1) as otbig,
                    tc.tile_pool(name="g_tmp", bufs=2) as g_tmp,
                ):
                    # w_dense.T half 1 on the sync ring, emitted before the
                    # otf reads: the sync queue is idle once a2a staging is
                    # done, so these stream during the AllToAll instead of
                    # waiting for the scalar queue to drain attention compute
                    wdh1 = []
                    for rh in range(2):
                        tw = wdto_pool.tile([P, 8, H // 2], F32, tag="wdto")
                        _dma_a(nc, tw,
                               wdTo[rh * 1024:(rh + 1) * 1024, 0:1024]
                               .rearrange("(c p) h -> p c h", p=P))
                        wdh1.append(tw)
                    otf = [None] * OC
                    for hp in range(2):
                        tlo = otbig.tile([P, 8, TOK], F32,
                                         tag=f"otl{hp}", name=f"otl{hp}")
                        _dma_a(nc, tlo, a2a1_out[hp][0:1024, :]
                               .rearrange("(c p) t -> p c t", p=P))
                        thi = otbig.tile([P, 8, TOK], F32,
                                         tag=f"oth{hp}", name=f"oth{hp}")
                        _dma_a(nc, thi, a2a1_out[hp][1024:2048, :]
                               .rearrange("(c p) t -> p c t", p=P))
                        for k in range(NKV):
                            for lh in range(2):
                                oc = k * 4 + 2 * hp + lh
                                to = otf_pool.tile([P, TOK], F32, tag="otf")
                                nc.vector.tensor_add(
                                    to.bitcast(F32R),
                                    tlo[:, 2 * k + lh, :],
                                    thi[:, 2 * k + lh, :])
                                otf[oc] = to
                    ho = [g_tmp.tile([P, H], F32, tag="ho", name=f"ho{i}")
                          for i in range(TC)]
                    for tcc in range(TC):
                        _dma_a(nc, ho[tcc],
                               hid_own[tcc * P:(tcc + 1) * P, :])
                    for half in range(2):
                        if half == 0:
                            wda = wdh1
                        else:
                            wda = []
                            for rh in range(2):
                                tw = wdto_pool.tile([P, 8, H // 2], F32,
                                                    tag="wdto")
                                _dma_a(nc, tw,
                                       wdTo[rh * 1024:(rh + 1) * 1024,
                                            1024:2048]
                                       .rearrange("(c p) h -> p c h", p=P))
                                wda.append(tw)
                        wdto_t = [wda[oc // 8][:, oc % 8, :]
                                  for oc in range(OC)]
                        oc_order = [c for c in range(OC) if c % 4 < 2] + \
                                   [c for c in range(OC) if c % 4 >= 2]
                        for tcc in range(TC):
                            for ntl in range(2):
                                nt = half * 2 + ntl
                                pd = ps_mm.tile([P, 512], F32, tag="mm")
                                for j, oc in enumerate(oc_order):
                                    _mm(nc, pd,
                                        otf[oc][:, tcc * P:(tcc + 1) * P],
                                        wdto_t[oc][:, ntl * 512:
                                                   (ntl + 1) * 512],
                                        start=(j == 0), stop=(j == OC - 1))
                                nc.vector.tensor_add(
                                    hidden2[tcc][:, nt * 512:(nt + 1) * 512],
                                    pd,
                                    ho[tcc][:, nt * 512:(nt + 1) * 512])

                    # ln2 + transpose to x2T_own
                    for tcc in range(TC):
                        sq = g_tmp.tile([P, H], F32, tag="sqg")
                        nc.vector.tensor_mul(sq, hidden2[tcc], hidden2[tcc])
                        ssum = g_tmp.tile([P, 1], F32, tag="ssum")
                        nc.vector.reduce_sum(ssum, sq,
                                             axis=mybir.AxisListType.X)
                        rms2 = g_tmp.tile([P, 1], F32, tag="rms2")
                        nc.scalar.activation(rms2, ssum, ACT.Sqrt,
                                             bias=epsc, scale=invH_col)
                        r2 = g_tmp.tile([P, 1], F32, tag="r2")
                        nc.vector.reciprocal(r2, rms2)
                        x2n = g_tmp.tile([P, H], F32, tag="x2n")
                        nc.scalar.activation(x2n, hidden2[tcc], ACT.Copy,
                                             scale=r2)
                        for hc in range(HC):
                            pt = ps_tr.tile([P, P], F32, tag="tr")
                            nc.tensor.transpose(
                                pt, x2n[:, hc * P:(hc + 1) * P], ident)
                            nc.scalar.activation(
                                x2T_own[hc][:, tcc * P:(tcc + 1) * P]
                                .bitcast(F32R), pt, ACT.Copy)
                    for hc in range(HC):
                        nc.sync.dma_start(out=ag_in[hc * P:(hc + 1) * P, :],
                                          in_=x2T_own[hc])

                # ---- routing on own tokens (group-limited top-2) ----
                with tc.tile_pool(name="r_tmp", bufs=2) as r_tmp:
                    gw = r_tmp.tile([P, HC, E], F32, tag="gw")
                    nc.sync.dma_start(
                        out=gw.bitcast(F32R),
                        in_=gate_wT.rearrange("(c p) e -> p c e",
                                              p=P).bitcast(F32R))
                    gb = r_tmp.tile([P, E], F32, tag="gb")
                    nc.sync.dma_start(out=gb, in_=gate_b.to_broadcast((P, E)))
                    for tcc in range(TC):
                        pl = ps_tr.tile([P, E], F32, tag="tr")
                        for hc in range(HC):
                            _mm(nc, pl, x2T_own[hc][:, tcc * P:(tcc + 1) * P],
                                gw[:, hc, :], start=(hc == 0),
                                stop=(hc == HC - 1))
                        ssig = r_tmp.tile([P, E], F32, tag="ssig")
                        nc.scalar.activation(ssig, pl, ACT.Sigmoid)
                        sb = r_tmp.tile([P, E], F32, tag="sbt")
                        nc.vector.tensor_add(sb, ssig, gb)
                        sbg = sb.rearrange("p (g two) -> p g two", two=2)
                        g4 = r_tmp.tile([P, NG], F32, tag="g4")
                        nc.vector.tensor_add(g4, sbg[:, :, 0], sbg[:, :, 1])
                        m1 = r_tmp.tile([P, 1], F32, tag="m1")
                        nc.vector.reduce_max(m1, g4, axis=mybir.AxisListType.X)
                        eq1 = r_tmp.tile([P, NG], F32, tag="eq1")
                        nc.vector.tensor_scalar(eq1, g4, m1, -BIG,
                                                mybir.AluOpType.is_equal,
                                                mybir.AluOpType.mult)
                        g4b = r_tmp.tile([P, NG], F32, tag="g4b")
                        nc.vector.tensor_add(g4b, g4, eq1)
                        m2 = r_tmp.tile([P, 1], F32, tag="m2")
                        nc.vector.reduce_max(m2, g4b,
                                             axis=mybir.AxisListType.X)
                        gmask = r_tmp.tile([P, NG], F32, tag="gmask")
                        nc.vector.tensor_scalar(gmask, g4, m2, None,
                                                mybir.AluOpType.is_ge)
                        pen = r_tmp.tile([P, E], F32, tag="pen")
                        peng = pen.rearrange("p (g two) -> p g two", two=2)
                        nc.vector.tensor_scalar(peng[:, :, 0], gmask, BIG,
                                                -BIG, mybir.AluOpType.mult,
                                                mybir.AluOpType.add)
                        nc.vector.tensor_scalar(peng[:, :, 1], gmask, BIG,
                                                -BIG, mybir.AluOpType.mult,
                                                mybir.AluOpType.add)
                        ms = r_tmp.tile([P, E], F32, tag="ms")
                        nc.vector.tensor_add(ms, sb, pen)
                        t1 = r_tmp.tile([P, 1], F32, tag="t1")
                        nc.vector.reduce_max(t1, ms, axis=mybir.AxisListType.X)
                        eq2 = r_tmp.tile([P, E], F32, tag="eq2")
                        nc.vector.tensor_scalar(eq2, ms, t1, -BIG,
                                                mybir.AluOpType.is_equal,
                                                mybir.AluOpType.mult)
                        ms2 = r_tmp.tile([P, E], F32, tag="ms2")
                        nc.vector.tensor_add(ms2, ms, eq2)
                        t2r = r_tmp.tile([P, 1], F32, tag="t2r")
                        nc.vector.reduce_max(t2r, ms2,
                                             axis=mybir.AxisListType.X)
                        sel = r_tmp.tile([P, E], F32, tag="sel")
                        nc.vector.tensor_scalar(sel, ms, t2r, None,
                                                mybir.AluOpType.is_ge)
                        wsel = r_tmp.tile([P, E], F32, tag="wsel")
                        nc.vector.tensor_mul(wsel, ssig, sel)
                        dsum = r_tmp.tile([P, 1], F32, tag="dsum")
                        nc.vector.reduce_sum(dsum, wsel,
                                             axis=mybir.AxisListType.X)
                        nc.vector.tensor_scalar_add(dsum, dsum, 1e-20)
                        rdsum = r_tmp.tile([P, 1], F32, tag="rdsum")
                        nc.vector.reciprocal(rdsum, dsum)
                        route = r_tmp.tile([P, E], F32, tag="route")
                        nc.vector.tensor_scalar(route, wsel, rdsum, None,
                                                mybir.AluOpType.mult)
                        ptr = ps_tr.tile([E, P], F32, tag="tr")
                        nc.tensor.transpose(ptr, route, ident)
                        rT = r_tmp.tile([E, P], F32, tag="rTst")
                        nc.scalar.activation(rT, ptr, ACT.Copy)
                        nc.sync.dma_start(
                            out=ag_in[H:H + E, tcc * P:(tcc + 1) * P], in_=rT)

                # ===== pair AllGather of x2T + routeT =====
                nc.gpsimd.collective_compute(
                    "AllGather", mybir.AluOpType.bypass,
                    replica_groups=[[0, 1], [2, 3], [4, 5], [6, 7]],
                    ins=[ag_in.opt()], outs=[ag_out.opt()])

                # ---- shared expert on own tokens (overlaps the AllGather) --
                # hc-outer so streamed weight tiles are consumed immediately;
                # the 4 m-chunk PSUM accumulators stay live across the loop.
                with (
                    tc.tile_pool(name="sh_w", bufs=2) as sh_w,
                    tc.tile_pool(name="sh_wd", bufs=1) as sh_wd,
                    tc.tile_pool(name="sh_hs", bufs=1) as sh_hs,
                    tc.tile_pool(name="sh_tmp", bufs=4) as sh_tmp,
                ):
                    hs_t = [sh_hs.tile([P, TOK], F32, tag=f"hs{i}",
                                       name=f"hs{i}") for i in range(MC)]
                    wsga = []
                    for rh in range(2):
                        tg = sh_w.tile([P, 8, M], F32, tag="wsg")
                        _dma_w(nc, tg,
                               wsgT[rh * 1024:(rh + 1) * 1024, :]
                               .rearrange("(c p) m -> p c m", p=P))
                        wsga.append(tg)
                    psg = [ps_mm.tile([P, TOK], F32, tag="mm",
                                      name=f"shg{mc}") for mc in range(MC)]
                    for hc in range(HC):
                        tg = wsga[hc // 8][:, hc % 8, :]
                        for mc in range(MC):
                            _mm(nc, psg[mc], tg[:, mc * P:(mc + 1) * P],
                                x2T_own[hc], start=(hc == 0),
                                stop=(hc == HC - 1))
                    sg_t = []
                    for mc in range(MC):
                        sg = sh_tmp.tile([P, TOK], F32, tag="sg",
                                         name=f"shsg{mc}")
                        nc.scalar.activation(sg, psg[mc], ACT.Silu)
                        sg_t.append(sg)
                    wsua = []
                    for rh in range(2):
                        tu = sh_w.tile([P, 8, M], F32, tag="wsu")
                        _dma_w(nc, tu,
                               wsuT[rh * 1024:(rh + 1) * 1024, :]
                               .rearrange("(c p) m -> p c m", p=P))
                        wsua.append(tu)
                    psu = [(ps_tr if mc < 2 else ps_mm)
                           .tile([P, TOK], F32,
                                 tag=("tr" if mc < 2 else "mm"),
                                 name=f"shu{mc}") for mc in range(MC)]
                    for hc in range(HC):
                        tu = wsua[hc // 8][:, hc % 8, :]
                        for mc in range(MC):
                            _mm(nc, psu[mc], tu[:, mc * P:(mc + 1) * P],
                                x2T_own[hc], start=(hc == 0),
                                stop=(hc == HC - 1))
                    for mc in range(MC):
                        nc.vector.tensor_mul(hs_t[mc].bitcast(F32R),
                                             psu[mc], sg_t[mc])
                    # shared down; base = hidden2 + shared_down (in place)
                    wsda = sh_wd.tile([P, MC, H], F32, tag="wsd",
                                      name="wsda")
                    _dma_w(nc, wsda,
                           wsdT.rearrange("(c p) h -> p c h", p=P))
                    wsd_t = [wsda[:, mc, :] for mc in range(MC)]
                    for tcc in range(TC):
                        for nt in range(NT):
                            pd = ps_mm.tile([P, 512], F32, tag="mm")
                            for mc in range(MC):
                                _mm(nc, pd,
                                    hs_t[mc][:, tcc * P:(tcc + 1) * P],
                                    wsd_t[mc][:, nt * 512:(nt + 1) * 512],
                                    start=(mc == 0), stop=(mc == MC - 1))
                            nc.vector.tensor_add(
                                hidden2[tcc][:, nt * 512:(nt + 1) * 512],
                                pd,
                                hidden2[tcc][:, nt * 512:(nt + 1) * 512])
                _x2_ctx.close()

                # ---- Phase K: my 4 experts over all 512 pair tokens ----
                with tc.tile_pool(name="khe", bufs=1) as khe:
                  with (
                    tc.tile_pool(name="kx", bufs=1) as kx,
                    tc.tile_pool(name="kw", bufs=2) as kw,
                    tc.tile_pool(name="ktmp", bufs=4) as ktmp,
                  ):
                    x2a = kx.tile([P, HC, PTOK], F32, tag="x2a", name="x2a")
                    for q in range(4):
                        for blk in range(2):
                            base_r = blk * AGROWS + q * 512
                            _dma_a(nc,
                                   x2a[:, q * 4:(q + 1) * 4,
                                       blk * TOK:(blk + 1) * TOK],
                                   ag_out[base_r:base_r + 512, :]
                                   .rearrange("(c p) t -> p c t", p=P))
                    x2T_all = [x2a[:, hc, :] for hc in range(HC)]
                    rT_all = kx.tile([E, PTOK], F32, tag="rta", name="rta")
                    for blk in range(2):
                        base_r = blk * AGROWS
                        _dma_a(nc, rT_all[:, blk * TOK:(blk + 1) * TOK],
                               ag_out[base_r + H:base_r + H + E, :])
                    es = kx.tile([E, EL], F32, tag="es", name="es")
                    _dma_a(nc, es, esel[:, :])
                    rpb = []
                    for el in range(EL):
                        prow = ps_row.tile([1, PTOK], F32, tag="row1")
                        _mm(nc, prow, es[:, el:el + 1], rT_all,
                            start=True, stop=True)
                        rrow = kx.tile([1, PTOK], F32, tag=f"rr{el}",
                                       name=f"rr{el}")
                        nc.scalar.activation(rrow.bitcast(F32R), prow,
                                             ACT.Copy)
                        pbb = ps_tr.tile([P, PTOK], F32, tag="tr")
                        _mm(nc, pbb, ones_row, rrow, start=True, stop=True)
                        rb = kx.tile([P, PTOK], F32, tag=f"rpb{el}",
                                     name=f"rpb{el}")
                        nc.scalar.activation(rb, pbb, ACT.Copy)
                        rpb.append(rb)

                    he = [[khe.tile([P, PTOK], F32, tag=f"he{el}_{mc}",
                                    name=f"he{el}_{mc}")
                           for mc in range(MC)] for el in range(EL)]
                    for el in range(EL):
                        psg = [ps_mm.tile([P, PTOK], F32, tag="mm",
                                          name=f"kg{el}_{mc}")
                               for mc in range(MC)]
                        wga = []
                        for rh in range(2):
                            tg = kw.tile([P, 8, M], F32, tag="wg")
                            _dma_a(nc, tg,
                                   wg_all[el * H + rh * 1024:
                                          el * H + (rh + 1) * 1024, :]
                                   .rearrange("(c p) m -> p c m", p=P))
                            wga.append(tg)
                        for hc in range(HC):
                            tg = wga[hc // 8][:, hc % 8, :]
                            for mc in range(MC):
                                _mm(nc, psg[mc], tg[:, mc * P:(mc + 1) * P],
                                    x2T_all[hc], start=(hc == 0),
                                    stop=(hc == HC - 1))
                        sg_t = []
                        for mc in range(MC):
                            sg = ktmp.tile([P, PTOK], F32, tag="ksg",
                                           name=f"ksg{el}_{mc}")
                            nc.scalar.activation(sg, psg[mc], ACT.Silu)
                            sg_t.append(sg)
                        psu = [(ps_tr if mc < 2 else ps_mm)
                               .tile([P, PTOK], F32,
                                     tag=("tr" if mc < 2 else "mm"),
                                     name=f"ku{el}_{mc}")
                               for mc in range(MC)]
                        wua = []
                        for rh in range(2):
                            tu = kw.tile([P, 8, M], F32, tag="wu")
                            _dma_a(nc, tu,
                                   wu_all[el * H + rh * 1024:
                                          el * H + (rh + 1) * 1024, :]
                                   .rearrange("(c p) m -> p c m", p=P))
                            wua.append(tu)
                        for hc in range(HC):
                            tu = wua[hc // 8][:, hc % 8, :]
                            for mc in range(MC):
                                _mm(nc, psu[mc], tu[:, mc * P:(mc + 1) * P],
                                    x2T_all[hc], start=(hc == 0),
                                    stop=(hc == HC - 1))
                        for mc in range(MC):
                            ur = ktmp.tile([P, PTOK], F32, tag="kur")
                            nc.vector.tensor_mul(ur, psu[mc], rpb[el])
                            nc.vector.tensor_mul(he[el][mc].bitcast(F32R),
                                                 ur, sg_t[mc])

                  # down: accumulate all 4 experts in PSUM per (tc, nt);
                  # stage the full [512, 2048] pair partial. K1's pools are
                  # closed at this point so the wdn stream has SBUF room.
                  if True:
                    with (
                        tc.tile_pool(name="kwd", bufs=2) as kwd,
                        tc.tile_pool(name="kst", bufs=4) as kst,
                    ):
                        for nt in range(NT):
                            wdna = kwd.tile([P, EL * MC, 512], F32,
                                            tag="wdn")
                            _dma_w(nc, wdna,
                                   wdn_all[:, nt * 512:(nt + 1) * 512]
                                   .rearrange("(c p) h -> p c h", p=P))
                            wdn_t = [wdna[:, i, :] for i in range(EL * MC)]
                            hf, ntl = divmod(nt, 2)
                            for tcc in range(PTC):
                                pd = ps_mm.tile([P, 512], F32, tag="mm")
                                for i in range(EL * MC):
                                    el, mc = divmod(i, MC)
                                    _mm(nc, pd,
                                        he[el][mc][:, tcc * P:(tcc + 1) * P],
                                        wdn_t[i],
                                        start=(i == 0), stop=(i == EL * MC - 1))
                                st = kst.tile([P, 512], F32, tag="mst")
                                nc.scalar.activation(st, pd, ACT.Copy)
                                nc.sync.dma_start(
                                    out=rs2_in[hf][tcc * P:(tcc + 1) * P,
                                                   ntl * 512:(ntl + 1) * 512],
                                    in_=st)
                            if nt == 1 or nt == 3:
                                # half columns complete: ReduceScatter them
                                # while the next half's down matmuls run
                                nc.gpsimd.collective_compute(
                                    "ReduceScatter", mybir.AluOpType.add,
                                    replica_groups=[[0, 1], [2, 3],
                                                    [4, 5], [6, 7]],
                                    ins=[rs2_in[hf].opt()],
                                    outs=[rs2_out[hf].opt()])

                # final: out = hidden2(+shared) + summed expert partials
                with tc.tile_pool(name="fin", bufs=4) as fin:
                    # per column-half: the first half's add + output write
                    # stream while the second ReduceScatter is still in flight
                    for hf in range(2):
                        for tcc in range(TC):
                            m0 = fin.tile([P, H // 2], F32, tag="mo")
                            nc.sync.dma_start(
                                out=m0,
                                in_=rs2_out[hf][tcc * P:(tcc + 1) * P, :])
                            fo = fin.tile([P, H // 2], F32, tag="fo")
                            nc.vector.tensor_add(
                                fo, m0,
                                hidden2[tcc][:, hf * 1024:(hf + 1) * 1024])
                            nc.sync.dma_start(
                                out=out[tcc * P:(tcc + 1) * P,
                                        hf * 1024:(hf + 1) * 1024], in_=fo)

    nc.compile()
    return nc


def prep_inputs(c, inputs):
    """Build the per-core input map (all host-side layout/weight-fold work)."""
    f = np.float32
    hs = np.asarray(inputs["hidden_states"], f)
    cos = np.asarray(inputs["cos"], f)
    sin = np.asarray(inputs["sin"], f)
    ln1 = np.asarray(inputs["ln1_w"], f)
    w_qkv = np.asarray(inputs["w_qkv"], f)
    q_ln = np.asarray(inputs["q_ln_w"], f)
    k_ln = np.asarray(inputs["k_ln_w"], f)
    w_dense = np.asarray(inputs["w_dense"], f)
    ln2 = np.asarray(inputs["ln2_w"], f)
    gate_w = np.asarray(inputs["gate_w"], f)
    gate_b = np.asarray(inputs["gate_bias"], f)
    we_gate = np.asarray(inputs["we_gate"], f)
    we_up = np.asarray(inputs["we_up"], f)
    we_down = np.asarray(inputs["we_down"], f)
    ws_gate = np.asarray(inputs["ws_gate"], f)
    ws_up = np.asarray(inputs["ws_up"], f)
    ws_down = np.asarray(inputs["ws_down"], f)

    b, kv, r = c // 4, c % 4, c % 2
    perm = np.concatenate([np.arange(64, 128), np.arange(64)])
    sign = np.concatenate([-np.ones(64, f), np.ones(64, f)])

    q_rows = np.arange(kv * QH * HD, (kv + 1) * QH * HD)
    k_rows = np.arange(NH * HD + kv * HD, NH * HD + (kv + 1) * HD)
    v_rows = np.arange(NH * HD + NKV * HD + kv * HD,
                       NH * HD + NKV * HD + (kv + 1) * HD)
    sel = np.concatenate([q_rows, k_rows, v_rows])
    w_sel = w_qkv[sel] * ln1[None, :]

    my_e = [EL * r + j for j in range(EL)]
    esel = np.zeros((E, EL), f)
    for j, e in enumerate(my_e):
        esel[e, j] = 1.0

    C = np.ascontiguousarray
    return {
        "hT": C(hs[b].T),
        "hid_own": C(hs[b, kv * TOK:(kv + 1) * TOK]),
        "cos_qw": C(cos[b].T * q_ln[:, None]),
        "sin_qw": C(sin[b].T * (q_ln[perm] * sign)[:, None]),
        "cos_kw": C(cos[b].T * k_ln[:, None]),
        "sin_kw": C(sin[b].T * (k_ln[perm] * sign)[:, None]),
        "wqkvT": C(w_sel.T),
        "wdTo": C(w_dense.T),
        "gate_wT": C((gate_w * ln2[None, :]).T),
        "gate_b": C(gate_b.reshape(1, E)),
        "esel": esel,
        "wg_all": C(np.concatenate(
            [(we_gate[e] * ln2[None, :]).T for e in my_e], axis=0)),
        "wu_all": C(np.concatenate(
            [(we_up[e] * ln2[None, :]).T for e in my_e], axis=0)),
        "wdn_all": C(np.concatenate(
            [we_down[e].T for e in my_e], axis=0)),
        "wsgT": C((ws_gate * ln2[None, :]).T),
        "wsuT": C((ws_up * ln2[None, :]).T),
        "wsdT": C(ws_down.T),
        "mlo": np.full((P, 1), 1.0 if b == 0 else 0.0, f),
        "mhi": np.full((P, 1), 1.0 if b == 1 else 0.0, f),
    }


_NC_CACHE = {}


def get_program():
    if "nc" not in _NC_CACHE:
        _NC_CACHE["nc"] = build_program()
    return _NC_CACHE["nc"]


def kernel(**inputs) -> np.ndarray:
    from concourse.bass_utils import run_bass_kernel_spmd

    nc = get_program()
    in_maps = [prep_inputs(c, inputs) for c in range(NCORES)]
    res = run_bass_kernel_spmd(nc, in_maps, core_ids=list(range(NCORES)))
    slices = [res.results[c]["out"] for c in range(NCORES)]
    full = np.concatenate(slices, axis=0)          # [2048, 2048] token-major
    return full.reshape(B, S, H).astype(np.float32)



# revision 84
# speedup vs baseline: 1.4406x; 1.4406x over previous
"""LLaDA2 MoE decoder layer as a single SPMD Bass/Tile kernel on 8 TRN2 cores.

v10 — 852us baseline -> ~592-620us measured (run-to-run fabric variance
~+/-30us, mostly collective durations).

Key techniques over the fp32 v2 baseline:
  - bf16 weights/activations everywhere except routing logits (a flipped
    top-k expert pick costs ~0.1 abs error -> full-fp32 matmul there) and
    the residual spine. bf16 does NOT change PE cycles (1 cyc/row for both
    f32r>=256-moving and bf16) but halves HBM traffic (109 -> 56MB/core),
    halves every collective payload, and halves SBUF footprints.
  - Emission order tuned for the in-order engine FIFOs: k head projected
    first, r1 square-work split across DVE+gpsimd off the critical path,
    r1 row-reduce after the qkv matmuls, per-head C+D (rms factor + rope)
    merged, softmax denominators accumulated with ones-column matmuls on
    the PE, softmax reciprocal via the DVE approx op (keeps the ACT queue
    free to run exps back-to-back; ACT function switches cost a ~1.3us
    table reload).
  - k_ln rms factor folded into the softmax exp's per-key activation
    scale, removing a broadcast multiply for the k head.
  - Weight streams (w_dense halves, shared/expert/down weights) emitted so
    their HWDGE descriptors are queued long before use; routing weights
    preloaded at t~0.
  - Shared expert emitted before routing so the PE fills the AllGather
    window; dense-projection accumulators spread across all 8 PSUM banks
    so the first-half chains run while the second AllToAll is in flight.

Sharding (unchanged from v2):
  - Attention: core c handles batch b = c//4, kv-group kv = c%4. Masked
    8-core AllToAll per head-pair (mlo/mhi zero-block trick, SPMD-safe)
    redistributes attention outputs to token owners.
  - MoE: expert-parallel degree 2 over core pairs. Pair AllGather of
    x2T (bf16) + routing rows, split in two so expert matmuls on the first
    half of the contraction start while the second half is in flight.
  - Expert partials return via 4 quarter-column ReduceScatters (bf16);
    each pair-RS has ~10us fixed cost, so 4 chunks is the sweet spot.

Numerics: absmax err 8.2e-3 on out scale 5.57 (gate budget ~0.11);
mean rel err 6.5e-3. All matmuls accumulate in fp32 PSUM.
"""

import math

import numpy as np

import concourse.bass as bass
import concourse.mybir as mybir
import concourse.tile as tile
from concourse import bacc
from concourse.masks import make_identity

# Problem shapes (hardcoded per contest rules).
B, S, H = 2, 1024, 2048
NH, NKV, HD = 16, 4, 128
E, TOPK, NG, TOPKG, M = 8, 2, 4, 2, 512
EPS = 1e-6
SCALE = HD ** -0.5
T = B * S
NCORES = 8
TOK = T // NCORES              # 256 tokens owned per core
QH = NH // NKV                 # 4 q heads per kv head
P = 128
F32 = mybir.dt.float32
F32R = mybir.dt.float32r
BF16 = mybir.dt.bfloat16
BIG = 1e30

HC = H // P                    # 16 h chunks
SC = S // P                    # 8 seq chunks per batch
MC = M // P                    # 4 m chunks
TC = TOK // P                  # 2 own-token chunks
NT = H // 512                  # 4 512-wide h' tiles
EL = 4                         # experts per core (EP=2 over pairs)
PTOK = 2 * TOK                 # 512 tokens per pair
PTC = PTOK // P                # 4 pair-token chunks
OC = NH * HD // P              # 16 o chunks (attention output dim)
AGR1 = H // 2                  # 1024 rows in first pair-AG (hc 0-7)
AGR2 = H // 2 + E              # 1032 rows in second pair-AG (hc 8-15 + route)


def r32(ap):
    return ap.bitcast(F32R)


def _mm(nc, out, lhsT, rhs, start, stop):
    nc.tensor.matmul(out, lhsT, rhs, start=start, stop=stop)


def _mmr(nc, out, lhsT, rhs, start, stop):
    nc.tensor.matmul(out, r32(lhsT), r32(rhs), start=start, stop=stop)


def _dma_a(nc, out, in_):
    # latency-critical / activation DMAs: sync (SP) HWDGE ring
    nc.sync.dma_start(out=out, in_=in_)


def _dma_w(nc, out, in_):
    # bulk weight streams: scalar (ACT) HWDGE ring, so they never queue in
    # front of critical activation transfers
    nc.scalar.dma_start(out=out, in_=in_)


def build_program():
    nc = bacc.Bacc("TRN2", target_bir_lowering=False, debug=False,
                   num_devices=NCORES)

    def inp(name, shape, dt=BF16):
        return nc.dram_tensor(name, list(shape), dt, kind="ExternalInput").ap()

    hT = inp("hT", (H, S))                 # hidden[b].T (bf16)
    hid_own = inp("hid_own", (TOK, H), F32)  # own-token hidden slice
    cos_qw = inp("cos_qw", (HD, S))        # cos[b].T * q_ln_w
    sin_qw = inp("sin_qw", (HD, S))        # sin[b].T * q_ln_w[perm] * sign
    cos_kw = inp("cos_kw", (HD, S))
    sin_kw = inp("sin_kw", (HD, S))
    wqkvT = inp("wqkvT", (H, 6 * P))       # [h, 4q+k+v heads] * ln1_w fold
    wdTo = inp("wdTo", (NH * HD, H))       # w_dense.T (full)
    gate_wT = inp("gate_wT", (H, E), F32)  # gate_w.T * ln2_w fold (fp32!)
    gate_b = inp("gate_b", (1, E), F32)
    esel = inp("esel", (E, EL))            # one-hot cols of this core's experts
    wg_all = inp("wg_all", (EL * H, M))    # per-expert we_gate[e].T * ln2 fold
    wu_all = inp("wu_all", (EL * H, M))
    wdn_all = inp("wdn_all", (EL * M, H))  # per-expert we_down[e].T
    wsgT = inp("wsgT", (H, M))             # shared expert, ln2_w folded
    wsuT = inp("wsuT", (H, M))
    wsdT = inp("wsdT", (M, H))
    mlo = inp("mlo", (P, 1), F32)          # 1.0 on batch-0 cores else 0.0
    mhi = inp("mhi", (P, 1), F32)          # 1.0 on batch-1 cores else 0.0

    out = nc.dram_tensor("out", [TOK, H], F32, kind="ExternalOutput").ap()

    ACT = mybir.ActivationFunctionType
    LNS = float(math.log(SCALE))

    with tile.TileContext(nc) as tc:
        with (
            tc.tile_pool(name="dram", bufs=1, space="DRAM") as dram,
            tc.tile_pool(name="const", bufs=1) as const,
            tc.tile_pool(name="ps_row", bufs=2, space="PSUM") as ps_row,
            tc.tile_pool(name="ps_tr", bufs=2, space="PSUM") as ps_tr,
            tc.tile_pool(name="ps_mm", bufs=4, space="PSUM") as ps_mm,
        ):
            # DRAM bounce buffers for collectives (all bf16)
            a2a1_in = [dram.tile([NH * HD, TOK], BF16, name=f"a2a1i{i}")
                       for i in range(2)]
            a2a1_out = [dram.tile([NH * HD, TOK], BF16, name=f"a2a1o{i}")
                        for i in range(2)]
            ag_in1 = dram.tile([AGR1, TOK], BF16)
            ag_out1 = dram.tile([2 * AGR1, TOK], BF16)
            ag_in2 = dram.tile([AGR2, TOK], BF16)
            ag_out2 = dram.tile([2 * AGR2, TOK], BF16)
            # pair-ReduceScatter has ~10us fixed overhead per op, so four
            # even chunks is the sweet spot (more chunks = more serial cc)
            RSC = [(0, 512), (512, 512), (1024, 512), (1536, 512)]
            rs_in = [dram.tile([PTOK, w], BF16, name=f"rsi{i}")
                     for i, (_, w) in enumerate(RSC)]
            rs_out = [dram.tile([TOK, w], BF16, name=f"rso{i}")
                      for i, (_, w) in enumerate(RSC)]
            r1_dram = dram.tile([1, S], F32)

            cstage = const.tile([P, 1], F32)
            cstage_r = const.tile([1, P], F32)
            ones_col_b = const.tile([P, 1], BF16)
            nc.vector.memset(cstage, 1.0)
            nc.scalar.activation(ones_col_b, cstage, ACT.Copy)
            ones_row_b = const.tile([1, P], BF16)
            nc.vector.memset(cstage_r, 1.0)
            nc.scalar.activation(ones_row_b, cstage_r, ACT.Copy)

            invH_col_b = const.tile([P, 1], BF16)
            nc.vector.memset(cstage, 1.0 / H)
            nc.scalar.activation(invH_col_b, cstage, ACT.Copy)
            invHD_col_b = const.tile([P, 1], BF16)
            nc.vector.memset(cstage, 1.0 / HD)
            nc.scalar.activation(invHD_col_b, cstage, ACT.Copy)
            invH_col = const.tile([P, 1], F32)
            nc.vector.memset(invH_col, 1.0 / H)
            epsc = const.tile([P, 1], F32)
            nc.vector.memset(epsc, EPS)
            lnsc = const.tile([P, 1], F32)
            nc.vector.memset(lnsc, LNS)
            zeroc = const.tile([P, 1], F32)
            nc.vector.memset(zeroc, 0.0)
            ident = const.tile([P, P], F32)
            make_identity(nc, ident)
            ident_b = const.tile([P, P], BF16)
            nc.scalar.activation(ident_b, ident, ACT.Copy)

            # routing weights preloaded at t~0 (they were arriving behind
            # the AllGather staging writes on the sync ring, delaying the
            # whole MoE midsection by ~40us)
            gw = const.tile([P, HC, E], F32, name="gw")
            gb = const.tile([P, E], F32, name="gb")



            # ============== ATTENTION (batch b, kv group kv) ==============
            # Emission strategy: k head first (every unit needs it), then q
            # heads in order; r1 squares split across DVE+gpsimd so neither
            # serializes; r_k (k_ln rms factor) is folded into the softmax
            # exp's per-partition scale instead of a broadcast multiply.
            with (
                tc.tile_pool(name="att_keep", bufs=1) as att_keep,
            ):
                qk_tiles = [att_keep.tile([P, S], BF16, tag=f"qk{i}",
                                          name=f"qk{i}") for i in range(5)]
                vT = att_keep.tile([P, S], BF16, tag="vT", name="vT")
                v_nat = [att_keep.tile([P, P], BF16, tag=f"vn{i}",
                                       name=f"vn{i}") for i in range(SC)]
                r1 = att_keep.tile([1, S], F32, tag="r1", name="r1")
                r1T = att_keep.tile([P, SC], F32, tag="r1T", name="r1T")
                r_heads = [att_keep.tile([1, S], BF16, tag=f"r_head{i}",
                                         name=f"r_head{i}")
                           for i in range(5)]
                rkT = att_keep.tile([P, SC], F32, tag="rkT", name="rkT")
                oT = [att_keep.tile([P, S], BF16, tag=f"oT{i}",
                                    name=f"oT{i}") for i in range(QH)]
                rk_dram = dram.tile([1, S], F32)

                from contextlib import ExitStack as _ES0
                _sq_ctx = _ES0()
                ab_sq = _sq_ctx.enter_context(tc.tile_pool(name="ab_sq",
                                                           bufs=1))
                _ab_ctx = _ES0()
                ab_h = _ab_ctx.enter_context(tc.tile_pool(name="ab_h",
                                                          bufs=1))
                ab_w = _ab_ctx.enter_context(tc.tile_pool(name="ab_w",
                                                          bufs=4))
                # one big strided DMA per quarter-matrix: fewer queue
                # triggers and bigger bursts for the 16 SDMA engines
                wqa = []
                for rh in range(4):
                    twq = ab_w.tile([P, 4, 6 * P], BF16, tag="wq")
                    _dma_w(nc, twq,
                           wqkvT[rh * 512:(rh + 1) * 512, :]
                           .rearrange("(c p) o -> p c o", p=P))
                    wqa.append(twq)
                w_tiles = [wqa[hc // 4][:, hc % 4, :] for hc in range(HC)]
                # first column-half of w_dense.T streams right behind the
                # qkv weights on the scalar ring (needed at the dense
                # projection ~240us in; streaming now keeps it off the
                # critical path)
                wda01 = []
                for rowh in range(2):
                    tw = const.tile([P, 8, H // 2], BF16,
                                    name=f"wdtoA{rowh}")
                    _dma_w(nc, tw,
                           wdTo[rowh * 1024:(rowh + 1) * 1024, 0:1024]
                           .rearrange("(c p) h -> p c h", p=P))
                    wda01.append(tw)
                hta = []
                for rh in range(8):
                    th = ab_h.tile([P, 2, S], BF16, tag=f"hT{rh}")
                    _dma_a(nc, th,
                           hT[rh * 256:(rh + 1) * 256, :]
                           .rearrange("(c p) s -> p c s", p=P))
                    hta.append(th)
                h_tiles = [hta[hc // 2][:, hc % 2, :] for hc in range(HC)]
                # cos/sin after the hT chunks on the sync ring (not needed
                # until rope, ~40us in)
                cq = att_keep.tile([P, S], BF16, tag="cq", name="cq")
                sqs = att_keep.tile([P, S], BF16, tag="sq_", name="sq_")
                ck = att_keep.tile([P, S], BF16, tag="ck", name="ck")
                sk = att_keep.tile([P, S], BF16, tag="sk", name="sk")
                nc.sync.dma_start(out=cq, in_=cos_qw[:, :])
                nc.sync.dma_start(out=sqs, in_=sin_qw[:, :])
                nc.sync.dma_start(out=ck, in_=cos_kw[:, :])
                nc.sync.dma_start(out=sk, in_=sin_kw[:, :])
                nc.sync.dma_start(
                    out=gw,
                    in_=gate_wT.rearrange("(c p) e -> p c e", p=P))
                nc.sync.dma_start(out=gb, in_=gate_b.to_broadcast((P, E)))

                # r1 squares first in both elementwise queues (inputs ready
                # as soon as the hT chunks land; nothing else blocks them).
                # gpsimd is ~1.6x slower per element, so it takes 10 and
                # the DVE 6 — the DVE must be free again by ~40us for rope.
                sq_r1 = []
                for hc in range(HC):
                    sq = ab_sq.tile([P, S], BF16, tag=f"sq{hc}",
                                    name=f"sqr{hc}")
                    eng = nc.vector if hc % 2 == 0 else nc.gpsimd
                    eng.tensor_mul(sq, h_tiles[hc], h_tiles[hc])
                    sq_r1.append(sq)

                # qkv projections, k head first, then q0.. so phase C/D of
                # the early heads overlaps the rest of the projection
                ps_r1 = [ps_row.tile([1, 512], F32, tag="row1",
                                     name=f"psr1_{_i}")
                         for _i in range(2)]
                for oc in (4, 0, 1, 5, 2, 3):
                    dst = qk_tiles[oc] if oc < 5 else vT
                    for t2 in range(2):
                        pq = ps_mm.tile([P, 512], F32, tag="mm")
                        for hc in range(HC):
                            _mm(nc, pq,
                                w_tiles[hc][:, oc * P:(oc + 1) * P],
                                h_tiles[hc][:, t2 * 512:(t2 + 1) * 512],
                                start=(hc == 0), stop=(hc == HC - 1))
                        nc.scalar.activation(
                            dst[:, t2 * 512:(t2 + 1) * 512], pq, ACT.Copy)
                lnr = ab_sq.tile([1, S], F32, tag="lnr")
                _ab_ctx.close()

                # ---- Phases C+D merged, per head, k first: rms factor,
                # then rope. r_k is folded into the softmax exp scale.
                with tc.tile_pool(name="cd_tmp", bufs=3) as cd_tmp:
                    for hh in (4, 0, 1, 2, 3):
                        # C: rms factor of this head (late heads have slack,
                        # so their squares go to the slower gpsimd)
                        sq = cd_tmp.tile([P, S], BF16, tag="sqc")
                        ceng = nc.vector if hh in (4, 0, 1) else nc.gpsimd
                        ceng.tensor_mul(sq, qk_tiles[hh], qk_tiles[hh])
                        lnt = cd_tmp.tile([1, S], F32, tag="lnt")
                        for t2 in range(2):
                            ps_rh = ps_row.tile([1, 512], F32, tag="row1")
                            _mm(nc, ps_rh,
                                invHD_col_b, sq[:, t2 * 512:(t2 + 1) * 512],
                                start=True, stop=True)
                            nc.scalar.activation(
                                lnt[:, t2 * 512:(t2 + 1) * 512],
                                ps_rh, ACT.Ln, bias=epsc[0:1])
                        # q heads get softmax 1/sqrt(HD) folded in via bias
                        if hh < 4:
                            nc.scalar.activation(r_heads[hh], lnt, ACT.Exp,
                                                 scale=-0.5, bias=lnsc[0:1])
                        else:
                            # r_k becomes the softmax exp's per-key scale
                            # (fp32 row, transposed to columns via DRAM)
                            rkrow = cd_tmp.tile([1, S], F32, tag="rkrow")
                            nc.scalar.activation(rkrow, lnt, ACT.Exp,
                                                 scale=-0.5)
                        # D: rope in place; the sin-side multiply runs on
                        # gpsimd so the DVE does two ops per head before
                        # the (q-only) r_q broadcast multiply
                        cw, sw = (cq, sqs) if hh < 4 else (ck, sk)
                        src = qk_tiles[hh]
                        swp = cd_tmp.tile([P, S], BF16, tag="swp")
                        nc.sync.dma_start(out=swp[0:64, :], in_=src[64:128, :])
                        nc.sync.dma_start(out=swp[64:128, :], in_=src[0:64, :])
                        ta = cd_tmp.tile([P, S], BF16, tag="ropeA")
                        seng = nc.vector if hh in (4, 0, 1) else nc.gpsimd
                        seng.tensor_mul(swp, swp, sw)
                        nc.vector.tensor_mul(ta, src, cw)
                        if hh == 4:
                            # k: no r_k multiply (folded into exp scale)
                            nc.vector.tensor_add(src, ta, swp)
                            nc.sync.dma_start(out=rk_dram, in_=rkrow)
                            nc.sync.dma_start(
                                out=rkT,
                                in_=rk_dram.rearrange("o (a p) -> (o p) a",
                                                      p=P))
                            continue
                        nc.vector.tensor_add(ta, ta, swp)
                        for t2 in range(2):
                            pb = ps_mm.tile([P, 512], F32, tag="mm",
                                            name=f"pbr{hh}_{t2}")
                            _mm(nc, pb, ones_row_b,
                                r_heads[hh][:, t2 * 512:(t2 + 1) * 512],
                                start=True, stop=True)
                            nc.vector.tensor_mul(
                                src[:, t2 * 512:(t2 + 1) * 512],
                                ta[:, t2 * 512:(t2 + 1) * 512], pb)

                # r1 reduction: inputs ready long ago; emitted here so the
                # tiny row matmuls (waiting on the slow gpsimd squares)
                # never stall the PE FIFO in front of qkv/rope/score work
                for hc in range(HC):
                    for t2 in range(2):
                        _mm(nc, ps_r1[t2],
                            invH_col_b,
                            sq_r1[hc][:, t2 * 512:(t2 + 1) * 512],
                            start=(hc == 0), stop=(hc == HC - 1))
                for t2 in range(2):
                    nc.scalar.activation(
                        lnr[:, t2 * 512:(t2 + 1) * 512],
                        ps_r1[t2], ACT.Ln, bias=epsc[0:1])
                nc.scalar.activation(r1, lnr, ACT.Exp, scale=-0.5)
                _sq_ctx.close()

                # r1/v_nat: row->column roundtrip, transpose, scaled copy
                nc.sync.dma_start(out=r1_dram, in_=r1)
                nc.sync.dma_start(
                    out=r1T,
                    in_=r1_dram.rearrange("o (a p) -> (o p) a", p=P))
                for sc in range(SC):
                    pt = ps_tr.tile([P, P], BF16, tag="tr")
                    nc.tensor.transpose(pt, vT[:, sc * P:(sc + 1) * P],
                                        ident_b)
                    nc.scalar.activation(v_nat[sc], pt,
                                         ACT.Copy, scale=r1T[:, sc:sc + 1])

                # ---- Phase E: attention per (head, t2), sw-pipelined ----
                # Each unit: 8 score MMs -> 8 scalar exps (bf16) -> 8 AV MMs
                # + an 8-MM ones-column chain for the softmax denominator
                # (PE replaces the old DVE tree-add). The denominator's
                # broadcast matmul + final scale of unit i are emitted during
                # unit i+1 so the PE FIFO never stalls on the scalar Ln/Exp.
                kT = qk_tiles[4]
                mlo_t = att_keep.tile([P, 1], F32, tag="mlo", name="mlo")
                mhi_t = att_keep.tile([P, 1], F32, tag="mhi", name="mhi")
                _dma_a(nc, mlo_t, mlo[:, :])
                _dma_a(nc, mhi_t, mhi[:, :])
                with (
                    tc.tile_pool(name="att_exp", bufs=4) as att_exp,
                    tc.tile_pool(name="att_row", bufs=4) as att_row,
                    tc.tile_pool(name="a2st", bufs=4) as a2st,
                ):
                    for hp in range(2):
                        for hh in (2 * hp, 2 * hp + 1):
                            qT = qk_tiles[hh]
                            pos, dens = [], []
                            for t2 in range(2):
                                ex = att_exp.tile([P, SC, 512], BF16,
                                                  tag="exp")
                                for sc in range(SC):
                                    pst = ps_mm.tile([P, 512], F32, tag="mm")
                                    _mm(nc, pst, kT[:, sc * P:(sc + 1) * P],
                                        qT[:, t2 * 512:(t2 + 1) * 512],
                                        start=True, stop=True)
                                    # r_k (k_ln rms factor) applied as the
                                    # per-key exp input scale
                                    nc.scalar.activation(
                                        ex[:, sc, :], pst, ACT.Exp,
                                        scale=rkT[:, sc:sc + 1])
                                po = ps_tr.tile([P, 512], F32, tag="tr")
                                for sc in range(SC):
                                    _mm(nc, po, v_nat[sc], ex[:, sc, :],
                                        start=(sc == 0), stop=(sc == SC - 1))
                                den = ps_row.tile([1, 512], F32, tag="row1")
                                for sc in range(SC):
                                    _mm(nc, den, ones_col_b, ex[:, sc, :],
                                        start=(sc == 0), stop=(sc == SC - 1))
                                pos.append(po)
                                dens.append(den)
                            # softmax reciprocal on the DVE (approx, 18
                            # bits — plenty): keeps the ACT queue free to
                            # run the next head's exps back to back with no
                            # table reloads
                            rdens = []
                            for t2 in range(2):
                                rden = att_row.tile([1, 512], F32,
                                                    tag="rden")
                                nc.vector.reciprocal_approx_fast(
                                    rden, dens[t2])
                                rdb = att_row.tile([1, 512], BF16,
                                                   tag="rdenb")
                                nc.vector.tensor_scalar_mul(rdb, rden, 1.0)
                                rdens.append(rdb)
                            for t2 in range(2):
                                pb2 = ps_row.tile([P, 512], F32, tag="row1",
                                                  name=f"pb2_{hh}_{t2}")
                                _mm(nc, pb2, ones_row_b, rdens[t2],
                                    start=True, stop=True)
                                rden_b = att_row.tile([P, 512], F32,
                                                      tag="rden_b")
                                nc.vector.tensor_scalar_mul(rden_b, pb2, 1.0)
                                nc.vector.tensor_mul(
                                    oT[hh][:, t2 * 512:(t2 + 1) * 512],
                                    pos[t2], rden_b)

                            # stage this head for its half-AllToAll right
                            # away (spreads the DVE mask-copies into the
                            # compute): slab for same-batch peer k goes to
                            # blocks k AND k+4, pre-scaled by the mlo/mhi
                            # per-batch masks so the cross-batch copy is
                            # exactly zero (SPMD-safe).
                            lh = hh - 2 * hp
                            for k in range(NKV):
                                slab = oT[hh][:, k * TOK:(k + 1) * TOK]
                                lo = a2st.tile([P, TOK], BF16, tag="stlo")
                                nc.vector.tensor_scalar_mul(lo, slab, mlo_t)
                                nc.scalar.dma_start(
                                    out=a2a1_in[hp][k * 256 + lh * P:
                                                    k * 256 + (lh + 1) * P,
                                                    :],
                                    in_=lo)
                                hi = a2st.tile([P, TOK], BF16, tag="sthi")
                                nc.vector.tensor_scalar_mul(hi, slab, mhi_t)
                                nc.scalar.dma_start(
                                    out=a2a1_in[hp][1024 + k * 256 + lh * P:
                                                    1024 + k * 256 +
                                                    (lh + 1) * P, :],
                                    in_=hi)
                        nc.gpsimd.collective_compute(
                            "AllToAll", mybir.AluOpType.bypass,
                            replica_groups=[[0, 1, 2, 3, 4, 5, 6, 7]],
                            ins=[a2a1_in[hp].opt()],
                            outs=[a2a1_out[hp].opt()])

            # ============ dense proj + ln2 + routing (own 256 tokens) ======
            from contextlib import ExitStack as _ES
            _x2_ctx = _ES()
            with (
                tc.tile_pool(name="keepG", bufs=1) as keepG,
            ):
                hidden2 = [keepG.tile([P, H], F32, tag=f"h2_{i}",
                                      name=f"h2_{i}") for i in range(TC)]
                x2keep = _x2_ctx.enter_context(
                    tc.tile_pool(name="x2keep", bufs=1))
                x2T_own = [x2keep.tile([P, TOK], F32, tag=f"x2o{i}",
                                       name=f"x2o{i}") for i in range(HC)]
                x2b = [x2keep.tile([P, TOK], BF16, tag=f"x2b{i}",
                                   name=f"x2b{i}") for i in range(HC)]

                with (
                    tc.tile_pool(name="wdto", bufs=1) as wdto_pool,
                    tc.tile_pool(name="otf", bufs=OC) as otf_pool,
                    tc.tile_pool(name="otbig", bufs=1) as otbig,
                    tc.tile_pool(name="g_tmp", bufs=2) as g_tmp,
                ):
                    wda = list(wda01)
                    for rowh in range(2):
                        tw = wdto_pool.tile([P, 8, H // 2], BF16,
                                            tag=f"wdtoB{rowh}",
                                            name=f"wdtoB{rowh}")
                        _dma_w(nc, tw,
                               wdTo[rowh * 1024:(rowh + 1) * 1024,
                                    1024:2048]
                               .rearrange("(c p) h -> p c h", p=P))
                        wda.append(tw)
                    otf = [None] * OC
                    for hp in range(2):
                        tlo = otbig.tile([P, 8, TOK], BF16,
                                         tag=f"otl{hp}", name=f"otl{hp}")
                        _dma_a(nc, tlo, a2a1_out[hp][0:1024, :]
                               .rearrange("(c p) t -> p c t", p=P))
                        thi = otbig.tile([P, 8, TOK], BF16,
                                         tag=f"oth{hp}", name=f"oth{hp}")
                        _dma_a(nc, thi, a2a1_out[hp][1024:2048, :]
                               .rearrange("(c p) t -> p c t", p=P))
                        for k in range(NKV):
                            for lh in range(2):
                                oc = k * 4 + 2 * hp + lh
                                to = otf_pool.tile([P, TOK], BF16, tag="otf")
                                nc.vector.tensor_add(
                                    to,
                                    tlo[:, 2 * k + lh, :],
                                    thi[:, 2 * k + lh, :])
                                otf[oc] = to
                    ho = [g_tmp.tile([P, H], F32, tag="ho", name=f"ho{i}")
                          for i in range(TC)]
                    for tcc in range(TC):
                        _dma_a(nc, ho[tcc],
                               hid_own[tcc * P:(tcc + 1) * P, :])
                    for half in range(2):
                        wdto_t = [wda[2 * half + oc // 8][:, oc % 8, :]
                                  for oc in range(OC)]
                        oc_order = [c for c in range(OC) if c % 4 < 2] + \
                                   [c for c in range(OC) if c % 4 >= 2]
                        for tcc in range(TC):
                            for ntl in range(2):
                                nt = half * 2 + ntl
                                # the 8 accumulators span all three PSUM
                                # pools so every chain's first-half (heads
                                # from a2a#0) runs while a2a#1 is in flight
                                pool, ptag = [(ps_mm, "mm"), (ps_mm, "mm"),
                                              (ps_tr, "tr"),
                                              (ps_row, "row1")][
                                                  2 * tcc + ntl]
                                pd = pool.tile([P, 512], F32, tag=ptag)
                                for j, oc in enumerate(oc_order):
                                    _mm(nc, pd,
                                        otf[oc][:, tcc * P:(tcc + 1) * P],
                                        wdto_t[oc][:, ntl * 512:
                                                   (ntl + 1) * 512],
                                        start=(j == 0), stop=(j == OC - 1))
                                nc.vector.tensor_add(
                                    hidden2[tcc][:, nt * 512:(nt + 1) * 512],
                                    pd,
                                    ho[tcc][:, nt * 512:(nt + 1) * 512])

                    # ln2 (fp32) -> x2T_own fp32 + bf16 copy; AG#1 fires
                    # as soon as the first 8 h-chunks are staged
                    x2n = [g_tmp.tile([P, H], F32, tag="x2n",
                                      name=f"x2n{i}") for i in range(TC)]
                    for tcc in range(TC):
                        sq = g_tmp.tile([P, H], F32, tag="sqg")
                        nc.vector.tensor_mul(sq, hidden2[tcc], hidden2[tcc])
                        ssum = g_tmp.tile([P, 1], F32, tag="ssum")
                        nc.vector.reduce_sum(ssum, sq,
                                             axis=mybir.AxisListType.X)
                        rms2 = g_tmp.tile([P, 1], F32, tag="rms2")
                        nc.scalar.activation(rms2, ssum, ACT.Sqrt,
                                             bias=epsc, scale=invH_col)
                        r2 = g_tmp.tile([P, 1], F32, tag="r2")
                        nc.vector.reciprocal(r2, rms2)
                        nc.scalar.activation(x2n[tcc], hidden2[tcc],
                                             ACT.Copy, scale=r2)
                    for hc in range(HC):
                        for tcc in range(TC):
                            pt = ps_tr.tile([P, P], F32, tag="tr")
                            nc.tensor.transpose(
                                pt, x2n[tcc][:, hc * P:(hc + 1) * P],
                                ident)
                            nc.scalar.activation(
                                x2T_own[hc][:, tcc * P:(tcc + 1) * P],
                                pt, ACT.Copy)
                        nc.vector.tensor_scalar_mul(x2b[hc], x2T_own[hc],
                                                    1.0)
                        if hc < 8:
                            nc.sync.dma_start(
                                out=ag_in1[hc * P:(hc + 1) * P, :],
                                in_=x2b[hc])
                        else:
                            nc.sync.dma_start(
                                out=ag_in2[(hc - 8) * P:(hc - 7) * P, :],
                                in_=x2b[hc])
                        if hc == 7:
                            nc.gpsimd.collective_compute(
                                "AllGather", mybir.AluOpType.bypass,
                                replica_groups=[[0, 1], [2, 3],
                                                [4, 5], [6, 7]],
                                ins=[ag_in1.opt()], outs=[ag_out1.opt()])

                # ---- routing on own tokens (group-limited top-2, fp32) ----
                with tc.tile_pool(name="r_tmp", bufs=2) as r_tmp:
                    for tcc in range(TC):
                        pl = ps_tr.tile([P, E], F32, tag="tr")
                        for hc in range(HC):
                            # full-precision fp32 matmul: the top-k pick is
                            # sensitive to sub-1e-3 logit error (f32r here
                            # flipped an expert choice on one token)
                            _mm(nc, pl,
                                x2T_own[hc][:, tcc * P:(tcc + 1) * P],
                                gw[:, hc, :], start=(hc == 0),
                                stop=(hc == HC - 1))
                        ssig = r_tmp.tile([P, E], F32, tag="ssig")
                        nc.scalar.activation(ssig, pl, ACT.Sigmoid)
                        sb = r_tmp.tile([P, E], F32, tag="sbt")
                        nc.vector.tensor_add(sb, ssig, gb)
                        sbg = sb.rearrange("p (g two) -> p g two", two=2)
                        g4 = r_tmp.tile([P, NG], F32, tag="g4")
                        nc.vector.tensor_add(g4, sbg[:, :, 0], sbg[:, :, 1])
                        m1 = r_tmp.tile([P, 1], F32, tag="m1")
                        nc.vector.reduce_max(m1, g4, axis=mybir.AxisListType.X)
                        eq1 = r_tmp.tile([P, NG], F32, tag="eq1")
                        nc.vector.tensor_scalar(eq1, g4, m1, -BIG,
                                                mybir.AluOpType.is_equal,
                                                mybir.AluOpType.mult)
                        g4b = r_tmp.tile([P, NG], F32, tag="g4b")
                        nc.vector.tensor_add(g4b, g4, eq1)
                        m2 = r_tmp.tile([P, 1], F32, tag="m2")
                        nc.vector.reduce_max(m2, g4b,
                                             axis=mybir.AxisListType.X)
                        gmask = r_tmp.tile([P, NG], F32, tag="gmask")
                        nc.vector.tensor_scalar(gmask, g4, m2, None,
                                                mybir.AluOpType.is_ge)
                        pen = r_tmp.tile([P, E], F32, tag="pen")
                        peng = pen.rearrange("p (g two) -> p g two", two=2)
                        nc.vector.tensor_scalar(peng[:, :, 0], gmask, BIG,
                                                -BIG, mybir.AluOpType.mult,
                                                mybir.AluOpType.add)
                        nc.vector.tensor_scalar(peng[:, :, 1], gmask, BIG,
                                                -BIG, mybir.AluOpType.mult,
                                                mybir.AluOpType.add)
                        ms = r_tmp.tile([P, E], F32, tag="ms")
                        nc.vector.tensor_add(ms, sb, pen)
                        t1 = r_tmp.tile([P, 1], F32, tag="t1")
                        nc.vector.reduce_max(t1, ms, axis=mybir.AxisListType.X)
                        eq2 = r_tmp.tile([P, E], F32, tag="eq2")
                        nc.vector.tensor_scalar(eq2, ms, t1, -BIG,
                                                mybir.AluOpType.is_equal,
                                                mybir.AluOpType.mult)
                        ms2 = r_tmp.tile([P, E], F32, tag="ms2")
                        nc.vector.tensor_add(ms2, ms, eq2)
                        t2r = r_tmp.tile([P, 1], F32, tag="t2r")
                        nc.vector.reduce_max(t2r, ms2,
                                             axis=mybir.AxisListType.X)
                        sel = r_tmp.tile([P, E], F32, tag="sel")
                        nc.vector.tensor_scalar(sel, ms, t2r, None,
                                                mybir.AluOpType.is_ge)
                        wsel = r_tmp.tile([P, E], F32, tag="wsel")
                        nc.vector.tensor_mul(wsel, ssig, sel)
                        dsum = r_tmp.tile([P, 1], F32, tag="dsum")
                        nc.vector.reduce_sum(dsum, wsel,
                                             axis=mybir.AxisListType.X)
                        nc.vector.tensor_scalar_add(dsum, dsum, 1e-20)
                        rdsum = r_tmp.tile([P, 1], F32, tag="rdsum")
                        nc.vector.reciprocal(rdsum, dsum)
                        route = r_tmp.tile([P, E], F32, tag="route")
                        nc.vector.tensor_scalar(route, wsel, rdsum, None,
                                                mybir.AluOpType.mult)
                        ptr = ps_tr.tile([E, P], F32, tag="tr")
                        nc.tensor.transpose(ptr, route, ident)
                        rT = r_tmp.tile([E, P], BF16, tag="rTst")
                        nc.scalar.activation(rT, ptr, ACT.Copy)
                        nc.sync.dma_start(
                            out=ag_in2[1024:1024 + E, tcc * P:(tcc + 1) * P],
                            in_=rT)

                # ===== second pair AllGather: x2T hc 8-15 + routeT =====
                nc.gpsimd.collective_compute(
                    "AllGather", mybir.AluOpType.bypass,
                    replica_groups=[[0, 1], [2, 3], [4, 5], [6, 7]],
                    ins=[ag_in2.opt()], outs=[ag_out2.opt()])

                # ---- shared expert on own tokens (overlaps the AllGathers)
                with (
                    tc.tile_pool(name="sh_w", bufs=2) as sh_w,
                    tc.tile_pool(name="sh_wd", bufs=1) as sh_wd,
                    tc.tile_pool(name="sh_hs", bufs=1) as sh_hs,
                    tc.tile_pool(name="sh_tmp", bufs=4) as sh_tmp,
                ):
                    hs_t = [sh_hs.tile([P, TOK], BF16, tag=f"hs{i}",
                                       name=f"hs{i}") for i in range(MC)]
                    wsga = []
                    for rh in range(2):
                        tg = sh_w.tile([P, 8, M], BF16, tag="wsg")
                        _dma_w(nc, tg,
                               wsgT[rh * 1024:(rh + 1) * 1024, :]
                               .rearrange("(c p) m -> p c m", p=P))
                        wsga.append(tg)
                    psg = [ps_mm.tile([P, TOK], F32, tag="mm",
                                      name=f"shg{mc}") for mc in range(MC)]
                    for hc in range(HC):
                        tg = wsga[hc // 8][:, hc % 8, :]
                        for mc in range(MC):
                            _mm(nc, psg[mc], tg[:, mc * P:(mc + 1) * P],
                                x2b[hc], start=(hc == 0),
                                stop=(hc == HC - 1))
                    sg_t = []
                    for mc in range(MC):
                        sg = sh_tmp.tile([P, TOK], BF16, tag="sg",
                                         name=f"shsg{mc}")
                        nc.scalar.activation(sg, psg[mc], ACT.Silu)
                        sg_t.append(sg)
                    wsua = []
                    for rh in range(2):
                        tu = sh_w.tile([P, 8, M], BF16, tag="wsu")
                        _dma_w(nc, tu,
                               wsuT[rh * 1024:(rh + 1) * 1024, :]
                               .rearrange("(c p) m -> p c m", p=P))
                        wsua.append(tu)
                    psu = [(ps_tr if mc < 2 else ps_row)
                           .tile([P, TOK], F32,
                                 tag=("tr" if mc < 2 else "row1"),
                                 name=f"shu{mc}") for mc in range(MC)]
                    for hc in range(HC):
                        tu = wsua[hc // 8][:, hc % 8, :]
                        for mc in range(MC):
                            _mm(nc, psu[mc], tu[:, mc * P:(mc + 1) * P],
                                x2b[hc], start=(hc == 0),
                                stop=(hc == HC - 1))
                    for mc in range(MC):
                        nc.vector.tensor_mul(hs_t[mc], psu[mc], sg_t[mc])
                    # shared down; base = hidden2 + shared_down (in place)
                    wsda = sh_wd.tile([P, MC, H], BF16, tag="wsd",
                                      name="wsda")
                    _dma_w(nc, wsda,
                           wsdT.rearrange("(c p) h -> p c h", p=P))
                    wsd_t = [wsda[:, mc, :] for mc in range(MC)]
                    for tcc in range(TC):
                        for nt in range(NT):
                            pd = ps_mm.tile([P, 512], F32, tag="mm")
                            for mc in range(MC):
                                _mm(nc, pd,
                                    hs_t[mc][:, tcc * P:(tcc + 1) * P],
                                    wsd_t[mc][:, nt * 512:(nt + 1) * 512],
                                    start=(mc == 0), stop=(mc == MC - 1))
                            nc.vector.tensor_add(
                                hidden2[tcc][:, nt * 512:(nt + 1) * 512],
                                pd,
                                hidden2[tcc][:, nt * 512:(nt + 1) * 512])
                _x2_ctx.close()

                # ---- Phase K: my 4 experts over all 512 pair tokens ----
                with tc.tile_pool(name="khe", bufs=1) as khe:
                    with (
                        tc.tile_pool(name="kx", bufs=1) as kx,
                        tc.tile_pool(name="kw", bufs=2) as kw,
                        tc.tile_pool(name="ktmp", bufs=4) as ktmp,
                    ):
                        # x2 of all pair tokens, split lo/hi so the hc 0-7
                        # matmuls only wait on AG#1
                        x2a_lo = kx.tile([P, 8, PTOK], BF16, tag="x2al",
                                         name="x2al")
                        x2a_hi = kx.tile([P, 8, PTOK], BF16, tag="x2ah",
                                         name="x2ah")
                        for q in range(2):
                            for blk in range(2):
                                base_r = blk * AGR1 + q * 512
                                _dma_a(nc,
                                       x2a_lo[:, q * 4:(q + 1) * 4,
                                              blk * TOK:(blk + 1) * TOK],
                                       ag_out1[base_r:base_r + 512, :]
                                       .rearrange("(c p) t -> p c t", p=P))
                        for q in range(2):
                            for blk in range(2):
                                base_r = blk * AGR2 + q * 512
                                _dma_a(nc,
                                       x2a_hi[:, q * 4:(q + 1) * 4,
                                              blk * TOK:(blk + 1) * TOK],
                                       ag_out2[base_r:base_r + 512, :]
                                       .rearrange("(c p) t -> p c t", p=P))
                        x2T_all = ([x2a_lo[:, i, :] for i in range(8)] +
                                   [x2a_hi[:, i, :] for i in range(8)])
                        rT_all = kx.tile([E, PTOK], BF16, tag="rta",
                                         name="rta")
                        for blk in range(2):
                            base_r = blk * AGR2 + 1024
                            _dma_a(nc, rT_all[:, blk * TOK:(blk + 1) * TOK],
                                   ag_out2[base_r:base_r + E, :])
                        es = kx.tile([E, EL], BF16, tag="es", name="es")
                        _dma_a(nc, es, esel[:, :])
                        rpb = []
                        for el in range(EL):
                            prow = ps_row.tile([1, PTOK], F32, tag="row1")
                            _mm(nc, prow, es[:, el:el + 1], rT_all,
                                start=True, stop=True)
                            rrow = kx.tile([1, PTOK], BF16, tag=f"rr{el}",
                                           name=f"rr{el}")
                            nc.scalar.activation(rrow, prow, ACT.Copy)
                            pbb = ps_tr.tile([P, PTOK], F32, tag="tr")
                            _mm(nc, pbb, ones_row_b, rrow,
                                start=True, stop=True)
                            rb = kx.tile([P, PTOK], BF16, tag=f"rpb{el}",
                                         name=f"rpb{el}")
                            nc.scalar.activation(rb, pbb, ACT.Copy)
                            rpb.append(rb)

                        he = [[khe.tile([P, PTOK], BF16, tag=f"he{el}_{mc}",
                                        name=f"he{el}_{mc}")
                               for mc in range(MC)] for el in range(EL)]
                        for el in range(EL):
                            psg = [ps_mm.tile([P, PTOK], F32, tag="mm",
                                              name=f"kg{el}_{mc}")
                                   for mc in range(MC)]
                            wga = []
                            for rh in range(2):
                                tg = kw.tile([P, 8, M], BF16, tag="wg")
                                _dma_w(nc, tg,
                                       wg_all[el * H + rh * 1024:
                                              el * H + (rh + 1) * 1024, :]
                                       .rearrange("(c p) m -> p c m", p=P))
                                wga.append(tg)
                            # last expert: run the gate as two mc-half
                            # chains so silu frees two PSUM banks a
                            # half-phase early — the down-projection's
                            # first accumulators then allocate while el3's
                            # up matmuls still run (was a ~14us stall)
                            mc_groups = ([(0, 1), (2, 3)] if el == EL - 1
                                         else [(0, 1, 2, 3)])
                            sg_t = [None] * MC
                            for grp in mc_groups:
                                for hc in range(HC):
                                    tg = wga[hc // 8][:, hc % 8, :]
                                    for mc in grp:
                                        _mm(nc, psg[mc],
                                            tg[:, mc * P:(mc + 1) * P],
                                            x2T_all[hc], start=(hc == 0),
                                            stop=(hc == HC - 1))
                                for mc in grp:
                                    sg = ktmp.tile([P, PTOK], BF16,
                                                   tag="ksg",
                                                   name=f"ksg{el}_{mc}")
                                    nc.scalar.activation(sg, psg[mc],
                                                         ACT.Silu)
                                    sg_t[mc] = sg
                            psu = [(ps_tr if mc < 2 else ps_row)
                                   .tile([P, PTOK], F32,
                                         tag=("tr" if mc < 2 else "row1"),
                                         name=f"ku{el}_{mc}")
                                   for mc in range(MC)]
                            wua = []
                            for rh in range(2):
                                tu = kw.tile([P, 8, M], BF16, tag="wu")
                                _dma_w(nc, tu,
                                       wu_all[el * H + rh * 1024:
                                              el * H + (rh + 1) * 1024, :]
                                       .rearrange("(c p) m -> p c m", p=P))
                                wua.append(tu)
                            for hc in range(HC):
                                tu = wua[hc // 8][:, hc % 8, :]
                                for mc in range(MC):
                                    _mm(nc, psu[mc],
                                        tu[:, mc * P:(mc + 1) * P],
                                        x2T_all[hc], start=(hc == 0),
                                        stop=(hc == HC - 1))
                            for mc in range(MC):
                                ur = ktmp.tile([P, PTOK], BF16, tag="kur")
                                nc.vector.tensor_mul(ur, psu[mc], rpb[el])
                                nc.vector.tensor_mul(he[el][mc], ur,
                                                     sg_t[mc])

                    # down: accumulate all 4 experts in PSUM per (tc, nt);
                    # quarter-column ReduceScatters overlap the remaining
                    # down matmuls so only the last one is exposed.
                    with (
                        tc.tile_pool(name="kwd", bufs=4) as kwd,
                        tc.tile_pool(name="kst", bufs=4) as kst,
                    ):
                        wdna = []
                        for nt in range(NT):
                            wt = kwd.tile([P, EL * MC, 512], BF16,
                                          tag="wdn", name=f"wdn{nt}")
                            _dma_w(nc, wt,
                                   wdn_all[:, nt * 512:(nt + 1) * 512]
                                   .rearrange("(c p) h -> p c h", p=P))
                            wdna.append(wt)
                        for ci, (off, w) in enumerate(RSC):
                            nt, noff = divmod(off, 512)
                            wdn_t = [wdna[nt][:, i, noff:noff + w]
                                     for i in range(EL * MC)]
                            for tcc in range(PTC):
                                pd = ps_mm.tile([P, w], F32, tag="mm")
                                for i in range(EL * MC):
                                    el, mc = divmod(i, MC)
                                    _mm(nc, pd,
                                        he[el][mc][:, tcc * P:(tcc + 1) * P],
                                        wdn_t[i],
                                        start=(i == 0),
                                        stop=(i == EL * MC - 1))
                                st = kst.tile([P, w], BF16, tag="mst")
                                nc.scalar.activation(st, pd, ACT.Copy)
                                nc.sync.dma_start(
                                    out=rs_in[ci][tcc * P:(tcc + 1) * P, :],
                                    in_=st)
                            nc.gpsimd.collective_compute(
                                "ReduceScatter", mybir.AluOpType.add,
                                replica_groups=[[0, 1], [2, 3],
                                                [4, 5], [6, 7]],
                                ins=[rs_in[ci].opt()],
                                outs=[rs_out[ci].opt()])

                # final: out = hidden2(+shared) + summed expert partials
                with tc.tile_pool(name="fin", bufs=4) as fin:
                    for ci, (off, w) in enumerate(RSC):
                        for tcc in range(TC):
                            m0 = fin.tile([P, w], BF16, tag="mo")
                            nc.sync.dma_start(
                                out=m0,
                                in_=rs_out[ci][tcc * P:(tcc + 1) * P, :])
                            fo = fin.tile([P, w], F32, tag="fo")
                            # final adds split across DVE and gpsimd (both
                            # SBUF-only here) so the last chunk's two
                            # token-tiles add in parallel
                            aeng = nc.vector if tcc == 0 else nc.gpsimd
                            aeng.tensor_add(
                                fo, m0,
                                hidden2[tcc][:, off:off + w])
                            nc.sync.dma_start(
                                out=out[tcc * P:(tcc + 1) * P,
                                        off:off + w], in_=fo)

    nc.compile()
    return nc


def prep_inputs(c, inputs):
    """Build the per-core input map (all host-side layout/weight-fold work)."""
    import ml_dtypes
    f = np.float32
    bf = ml_dtypes.bfloat16
    hs = np.asarray(inputs["hidden_states"], f)
    cos = np.asarray(inputs["cos"], f)
    sin = np.asarray(inputs["sin"], f)
    ln1 = np.asarray(inputs["ln1_w"], f)
    w_qkv = np.asarray(inputs["w_qkv"], f)
    q_ln = np.asarray(inputs["q_ln_w"], f)
    k_ln = np.asarray(inputs["k_ln_w"], f)
    w_dense = np.asarray(inputs["w_dense"], f)
    ln2 = np.asarray(inputs["ln2_w"], f)
    gate_w = np.asarray(inputs["gate_w"], f)
    gate_b = np.asarray(inputs["gate_bias"], f)
    we_gate = np.asarray(inputs["we_gate"], f)
    we_up = np.asarray(inputs["we_up"], f)
    we_down = np.asarray(inputs["we_down"], f)
    ws_gate = np.asarray(inputs["ws_gate"], f)
    ws_up = np.asarray(inputs["ws_up"], f)
    ws_down = np.asarray(inputs["ws_down"], f)

    b, kv, r = c // 4, c % 4, c % 2
    perm = np.concatenate([np.arange(64, 128), np.arange(64)])
    sign = np.concatenate([-np.ones(64, f), np.ones(64, f)])

    q_rows = np.arange(kv * QH * HD, (kv + 1) * QH * HD)
    k_rows = np.arange(NH * HD + kv * HD, NH * HD + (kv + 1) * HD)
    v_rows = np.arange(NH * HD + NKV * HD + kv * HD,
                       NH * HD + NKV * HD + (kv + 1) * HD)
    sel = np.concatenate([q_rows, k_rows, v_rows])
    w_sel = w_qkv[sel] * ln1[None, :]

    my_e = [EL * r + j for j in range(EL)]
    esel = np.zeros((E, EL), f)
    for j, e in enumerate(my_e):
        esel[e, j] = 1.0

    def C(x):
        return np.ascontiguousarray(x)

    def Cb(x):
        return np.ascontiguousarray(np.asarray(x, f).astype(bf))

    return {
        "hT": Cb(hs[b].T),
        "hid_own": C(hs[b, kv * TOK:(kv + 1) * TOK]),
        "cos_qw": Cb(cos[b].T * q_ln[:, None]),
        "sin_qw": Cb(sin[b].T * (q_ln[perm] * sign)[:, None]),
        "cos_kw": Cb(cos[b].T * k_ln[:, None]),
        "sin_kw": Cb(sin[b].T * (k_ln[perm] * sign)[:, None]),
        "wqkvT": Cb(w_sel.T),
        "wdTo": Cb(w_dense.T),
        "gate_wT": C((gate_w * ln2[None, :]).T),
        "gate_b": C(gate_b.reshape(1, E)),
        "esel": esel.astype(bf),
        "wg_all": Cb(np.concatenate(
            [(we_gate[e] * ln2[None, :]).T for e in my_e], axis=0)),
        "wu_all": Cb(np.concatenate(
            [(we_up[e] * ln2[None, :]).T for e in my_e], axis=0)),
        "wdn_all": Cb(np.concatenate(
            [we_down[e].T for e in my_e], axis=0)),
        "wsgT": Cb((ws_gate * ln2[None, :]).T),
        "wsuT": Cb((ws_up * ln2[None, :]).T),
        "wsdT": Cb(ws_down.T),
        "mlo": np.full((P, 1), 1.0 if b == 0 else 0.0, f),
        "mhi": np.full((P, 1), 1.0 if b == 1 else 0.0, f),
    }


_NC_CACHE = {}


def get_program():
    if "nc" not in _NC_CACHE:
        _NC_CACHE["nc"] = build_program()
    return _NC_CACHE["nc"]


def kernel(**inputs) -> np.ndarray:
    from concourse.bass_utils import run_bass_kernel_spmd

    nc = get_program()
    in_maps = [prep_inputs(c, inputs) for c in range(NCORES)]
    res = run_bass_kernel_spmd(nc, in_maps, core_ids=list(range(NCORES)))
    slices = [res.results[c]["out"] for c in range(NCORES)]
    full = np.concatenate(slices, axis=0)          # [2048, 2048] token-major
    return full.reshape(B, S, H).astype(np.float32)
